# revision 3
# baseline (speedup 1.0000x reference)
"""Trainium2 Bass kernel for the gnn_message_passing problem.

Math reduction: the reference collapses to

    out[b, k] = sum_c x[b, c] * weights[8191-k, c] + bias[8191-k]

i.e. a [32,1024] x [1024,1024]^T matmul + bias.  Shard the 1024 output
features row-wise across 8 cores (128 each); x replicated; no collectives.

v3: everything a core needs arrives in ONE bf16 DMA of [128, 1282]
(2564 B per partition, contiguous):
    cols    0..1023  weights, packed wt[p, n*128 + k'] = W[k', n*128 + p]
    cols 1024..1279  x,       packed xt[p, n*32  + b ] = x[b, n*128 + p]
    cols 1280..1281  f32 bias bit-pattern (partition p = bias for k'=p)
Per-DMA fixed cost dominates at this size, so one DMA beats any split.
The 8 PSUM-accumulated bf16 matmuls, then a single vector-engine
tensor_scalar_add moves PSUM->SBUF and adds the (bitcast f32) bias.
"""

import numpy as np
from ml_dtypes import bfloat16

import concourse.bacc as bacc
import concourse.bass as bass
import concourse.mybir as mybir
from concourse.bass_utils import run_bass_kernel_spmd
from concourse.tile import TileContext

# Under BASS_TRACE=1 + axon, run_bass_kernel_spmd imports antenv.axon_hooks
# unconditionally; some images ship an antenv stub without it.  Provide a
# null hook registry so tracing degrades gracefully instead of crashing.
try:
    import antenv.axon_hooks  # noqa: F401
except ImportError:  # pragma: no cover
    import sys as _sys
    import types as _types

    import antenv as _antenv

    _m = _types.ModuleType("antenv.axon_hooks")
    _m._hook = None
    _m.set_axon_ntff_profile_hook = lambda hook: setattr(_m, "_hook", hook)
    _m.get_axon_ntff_profile_hook = lambda: _m._hook
    _sys.modules["antenv.axon_hooks"] = _m
    _antenv.axon_hooks = _m

NODES = 8192
IN_F = 1024
OUT_F = 1024
B = 32
N_CORES = 8
KPC = OUT_F // N_CORES   # output features per core: 128
NCHUNK = IN_F // 128     # contraction chunks: 8
XOFF = IN_F              # col offset of packed x
BOFF = XOFF + NCHUNK * B # col offset of packed bias (2 bf16 cols = 1 f32)
COLS = BOFF + 2          # 1282

F32 = mybir.dt.float32
BF16 = mybir.dt.bfloat16

_NC = None
LAST_RESULT = None  # BassKernelResults of the most recent run (for profiling)


def _build_nc():
    nc = bacc.Bacc(None, target_bir_lowering=False)

    inp = nc.dram_tensor("inp", [128, COLS], BF16, kind="ExternalInput")
    out = nc.dram_tensor("out", [KPC, B], F32, kind="ExternalOutput")

    with TileContext(nc) as tc:
        with (
            tc.tile_pool(name="sbuf", bufs=1) as pool,
            tc.tile_pool(name="psum", bufs=1, space=bass.MemorySpace.PSUM) as psum_pool,
        ):
            it = pool.tile([128, COLS], BF16)
            o_t = pool.tile([KPC, B], F32)
            ps = psum_pool.tile([KPC, B], F32)

            nc.sync.dma_start(it[:], inp[:])

            for n in range(NCHUNK):
                nc.tensor.matmul(
                    ps[:],
                    it[:, n * KPC : (n + 1) * KPC],          # lhsT [c=128, k'=128]
                    it[:, XOFF + n * B : XOFF + (n + 1) * B],  # rhs [c=128, b=32]
                    start=(n == 0),
                    stop=(n == NCHUNK - 1),
                )

            # PSUM -> SBUF with per-partition f32 bias add on the vector engine.
            nc.vector.tensor_scalar_add(o_t[:], ps[:], it[:, BOFF : BOFF + 2].bitcast(F32))
            nc.sync.dma_start(out[:], o_t[:])

    nc.finalize()
    return nc


def _pack_inputs(x, weights, bias):
    """Returns [N_CORES, 128, COLS] uint16 (bf16 bit patterns + f32 bias halves)."""
    w_eff = weights[NODES - OUT_F :, :IN_F][::-1]  # [1024 (k), 1024 (c)]
    b_eff = bias[NODES - OUT_F :][::-1]            # [1024]

    buf = np.empty((N_CORES, 128, COLS), dtype=np.uint16)
    # weights: w_eff[(i,k'),(n,p)] -> buf[i][p, n*KPC + k']
    wt = w_eff.reshape(N_CORES, KPC, NCHUNK, 128).transpose(0, 3, 2, 1)
    buf[:, :, :XOFF] = (
        wt.reshape(N_CORES, 128, IN_F).astype(bfloat16).view(np.uint16)
    )
    # x: x[b, (n,p)] -> buf[i][p, XOFF + n*B + b], replicated
    xt = x.reshape(B, NCHUNK, 128).transpose(2, 1, 0).reshape(128, NCHUNK * B)
    buf[:, :, XOFF:BOFF] = xt.astype(bfloat16).view(np.uint16)[None]
    # bias: f32 little-endian halves; partition p carries bias for k'=p
    bh = np.ascontiguousarray(b_eff.astype(np.float32)).view(np.uint16)
    buf[:, :, BOFF:] = bh.reshape(N_CORES, KPC, 2)
    return buf


def kernel(x: np.ndarray, weights: np.ndarray, bias: np.ndarray) -> np.ndarray:
    global _NC, LAST_RESULT
    if _NC is None:
        _NC = _build_nc()

    x = np.asarray(x, dtype=np.float32)
    weights = np.asarray(weights, dtype=np.float32)
    bias = np.asarray(bias, dtype=np.float32)

    packed = _pack_inputs(x, weights, bias).view(bfloat16)
    in_maps = [{"inp": np.ascontiguousarray(packed[i])} for i in range(N_CORES)]

    LAST_RESULT = run_bass_kernel_spmd(_NC, in_maps, list(range(N_CORES)))

    # Gather: core i returns out[k', b] for k = i*KPC + k'.
    out_t = np.concatenate([r["out"] for r in LAST_RESULT.results], axis=0)
    return np.ascontiguousarray(out_t.T)
